# revision 3
# baseline (speedup 1.0000x reference)
"""AE (associative embedding) push/pull loss on 8 Trainium2 NeuronCores.

Data-parallel over the batch: core c handles images [4c, 4c+4). Per core the
kernel gathers the visible (person, joint) tag values out of the on-device
4x1114112 tag shard with 9 fixed-shape indirect (SWDGE) DMAs of up to 128
scattered f32 each (1152 slot grid; pad slots carry out-of-bounds indices
that the bounds check silently skips, so the mostly-pad last column
completes ~1us faster and there are no per-call NEFF variants). The TT tile
is zeroed by gpsimd before the gathers so skipped slots read as 0 and route
harmlessly to the unused person row 127.

Per-person sums of t (S1) and t^2 (S2) accumulate in two separate PSUM
groups via 18 single-column PE matmuls whose one-hot person-indicator
planes are built ON DEVICE by the idle DVE (is_equal of a static iota row
against per-slot person-id columns; consts DMA is ~210KB instead of 727KB,
which also removes the SDMA-15 backlog straggler). Splitting S1 from S2
takes the ScalarE squares off the critical path: the push chain (mu ->
PE broadcast-transpose onto a preloaded additive pair-mask PSUM bank ->
Square(scale=-1, bias=mu) -> Exp(accum_out=row sums) -> per-image matmul)
starts as soon as the last t-column matmul finishes, while the S2/pull
chain fills PE/DVE gaps.

Startup: the gidx (SP ring) and consts (ACT ring) input DMAs plus gpsimd's
TT memset, bounds register, g_sem wait and first two gather instructions
are hoisted into the entry block ahead of the barrier (gpsimd's barrier
drain is dropped -- its ordering is carried by the tcol semaphores), so the
Q7 ucode load and first descriptor generations overlap engine boot. The
32B result DMA is issued by the otherwise idle SP engine with no completion
wait; the end-of-block engine drains cover it.

Host-side work is index/mask preparation only (no tag data is touched).
Raw bacc (no TileContext): per-engine chain semaphores serialize
same-engine RAW hazards and double as cross-engine handshakes.
"""

from contextlib import ExitStack

import numpy as np

try:
    import concourse  # noqa: F401
except ImportError:
    import sys

    sys.path.insert(0, "/opt/trn_rl_repo")

from concourse import bacc, bass, mybir
from concourse.bass_utils import run_bass_kernel_spmd

N, M, K, KHW = 32, 30, 17, 1114112
NCORES = 8
IPC = N // NCORES
P = 128
PPI = IPC * M
C = 9  # gather columns; 1152 slots >= max visible (~1030, +5.8 sigma)

f32 = mybir.dt.float32
i32 = mybir.dt.int32
Alu = mybir.AluOpType
Act = mybir.ActivationFunctionType

BIG = 1.0e4  # additive mask magnitude: exp(-(BIG+d)^2) underflows to 0

# consts column layout
RC0, NRC0, PV0, PVN0, NI0, RD0, RN0 = 0, 1, 2, 3, 4, 5, 6
BM0 = 7
BI0 = BM0 + P
ID0 = BI0 + IPC
IO0 = ID0 + P
PID0 = IO0 + P
CC = PID0 + C


def build_nc():
    nc = bacc.Bacc(
        "TRN2",
        target_bir_lowering=False,
        debug=False,
        enable_asserts=False,
        num_devices=NCORES,
        detect_race_conditions=False,
    )
    tags_d = nc.declare_dram_parameter("tags", [IPC * KHW, 1], f32, isOutput=False)
    gidx_d = nc.declare_dram_parameter("gidx", [P, C], i32, isOutput=False)
    consts_d = nc.declare_dram_parameter("consts", [P, CC], f32, isOutput=False)
    out_d = nc.declare_dram_parameter("out", [IPC, 2], f32, isOutput=True)

    ctx = ExitStack()
    g_sem = ctx.enter_context(nc.semaphore("g_sem"))
    c_sem = ctx.enter_context(nc.semaphore("c_sem"))
    o_sem = ctx.enter_context(nc.semaphore("o_sem"))
    vc = ctx.enter_context(nc.semaphore("vc_sem"))
    tc = ctx.enter_context(nc.semaphore("tc_sem"))
    sc = ctx.enter_context(nc.semaphore("sc_sem"))
    tcol = [ctx.enter_context(nc.semaphore(f"tcol{c}")) for c in range(C)]

    gidx_sb = ctx.enter_context(nc.sbuf_tensor("gidx_sb", [P, C], i32))
    c_sb = ctx.enter_context(nc.sbuf_tensor("c_sb", [P, CC], f32))
    lh_sb = ctx.enter_context(nc.sbuf_tensor("lh_sb", [P, C * P], f32))
    TT = ctx.enter_context(nc.sbuf_tensor("TT", [P, 2 * C], f32))
    mu = ctx.enter_context(nc.sbuf_tensor("mu", [P, 1], f32))
    t2s = ctx.enter_context(nc.sbuf_tensor("t2s", [P, 1], f32))
    negp = ctx.enter_context(nc.sbuf_tensor("negp", [P, 1], f32))
    X = ctx.enter_context(nc.sbuf_tensor("X", [P, 2], f32))
    d2 = ctx.enter_context(nc.sbuf_tensor("d2", [P, P], f32))
    e = ctx.enter_context(nc.sbuf_tensor("e", [P, P], f32))
    pm = ctx.enter_context(nc.sbuf_tensor("pm", [P, P], f32))
    me = ctx.enter_context(nc.sbuf_tensor("me", [P, P], f32))
    res = ctx.enter_context(nc.sbuf_tensor("res", [IPC, 2], f32))
    warm = ctx.enter_context(nc.sbuf_tensor("warm", [1, 1], f32))
    muT_t = ctx.enter_context(nc.psum_tensor("muT", [P, 512], f32))
    pvT_t = ctx.enter_context(nc.psum_tensor("pvT", [P, 512], f32))
    fmT_t = ctx.enter_context(nc.psum_tensor("fmT", [P, 512], f32))
    fmS_t = ctx.enter_context(nc.psum_tensor("fmS", [P, 512], f32))
    fin_t = ctx.enter_context(nc.psum_tensor("fin", [IPC, 512], f32))

    rc_ap = c_sb.ap()[:, RC0 : RC0 + 1]
    negrc_ap = c_sb.ap()[:, NRC0 : NRC0 + 1]
    pv_ap = c_sb.ap()[:, PV0 : PV0 + 1]
    pvnb_ap = c_sb.ap()[:, PVN0 : PVN0 + 1]
    nim_ap = c_sb.ap()[0:IPC, NI0 : NI0 + 1]
    rd_ap = c_sb.ap()[0:IPC, RD0 : RD0 + 1]
    rn_ap = c_sb.ap()[0:IPC, RN0 : RN0 + 1]
    bmask = c_sb.ap()[:, BM0 : BM0 + P]
    binds = c_sb.ap()[:, BI0 : BI0 + IPC]
    ident = c_sb.ap()[:, ID0 : ID0 + P]
    iorow = c_sb.ap()[:, IO0 : IO0 + P]
    muT = muT_t.ap()[:, :P]
    pvT = pvT_t.ap()[:, :P]
    fmT = fmT_t.ap()[:, :1]
    fmS = fmS_t.ap()[:, :1]
    fin = fin_t.ap()[:, :2]

    V_WARM = 1
    V_LH = lambda b: 2 + b  # b = 0..C-1
    V_PM = 2 + C
    V_BIGM = 3 + C
    V_MU = 4 + C
    V_T2S = 5 + C
    V_NEGP = 6 + C
    V_X1 = 7 + C
    V_RES = 8 + C
    S_SQ = lambda c: 1 + c
    S_D2 = C + 1
    S_EXP = C + 2
    S_RES1 = C + 3

    T_PVT = 1
    T_MMT = lambda c: 3 + 2 * c      # S1 matmul for column c done
    T_MMS = lambda c: 4 + 2 * c if c < C - 1 else 2 * C + 3
    T_MUT = 2 * C + 2                # muT now precedes the last S2 matmul
    T_FIN = 2 * C + 4

    with nc.Block(no_gpsimd_drain=True) as block:

        @block.sync
        def _(sync):
            sync.dma_start(out=gidx_sb.ap(), in_=gidx_d[:]).then_inc(g_sem, 16)
            sync.wait_ge(sc, S_RES1)
            sync.wait_ge(vc, V_RES)
            sync.dma_start(out=out_d[:], in_=res.ap()).then_inc(o_sem, 16)

        @block.gpsimd
        def _(gpsimd):
            gpsimd.memset(TT.ap(), 0.0)
            breg = gpsimd.to_reg(IPC * KHW - 1)
            gpsimd.wait_ge(g_sem, 16)
            for c in range(C):
                gpsimd.indirect_dma_start(
                    out=TT.ap()[:, 2 * c : 2 * c + 1],
                    out_offset=None,
                    in_=tags_d[:],
                    in_offset=bass.IndirectOffsetOnAxis(
                        ap=gidx_sb.ap()[:, c : c + 1], axis=0
                    ),
                    bounds_check=breg,
                    oob_is_err=False,
                ).then_inc(tcol[c], 16)

        @block.vector
        def _(vector):
            vn = {"n": 0}

            def chain(instr):
                instr.then_inc(vc, 1)
                vn["n"] += 1
                return vn["n"]

            def W():
                vector.wait_ge(vc, vn["n"])

            # zero the scalar-engine warm-up scratch at t=0
            i = chain(vector.memset(warm.ap(), 0.0))
            assert i == V_WARM
            # build the one-hot person-indicator planes on device
            vector.wait_ge(c_sem, 16)
            for b in range(C):
                i = chain(vector.tensor_scalar(
                    out=lh_sb.ap()[:, P * b : P * (b + 1)], in0=iorow,
                    scalar1=c_sb.ap()[:, PID0 + b : PID0 + b + 1],
                    scalar2=None, op0=Alu.is_equal))
                assert i == V_LH(b)
            # additive pair mask BIGM = BIG * (1 - pv_p*pv_q*blk)
            vector.wait_ge(tc, T_PVT)
            i = chain(vector.scalar_tensor_tensor(
                out=pm.ap(), in0=pvT, scalar=pvnb_ap, in1=bmask,
                op0=Alu.mult, op1=Alu.mult))
            assert i == V_PM
            W()
            i = chain(vector.tensor_scalar(
                out=me.ap(), in0=pm.ap(), scalar1=BIG, scalar2=None,
                op0=Alu.add))
            assert i == V_BIGM
            # per-person stats straight from PSUM
            vector.wait_ge(tc, T_MMT(C - 1))
            i = chain(vector.tensor_mul(mu.ap(), fmT, rc_ap))
            assert i == V_MU
            vector.wait_ge(tc, T_MMS(C - 1))
            i = chain(vector.tensor_copy(t2s.ap(), fmS))
            assert i == V_T2S
            W()
            i = chain(vector.scalar_tensor_tensor(
                out=negp.ap(), in0=fmT, scalar=mu.ap(), in1=t2s.ap(),
                op0=Alu.mult, op1=Alu.subtract))
            assert i == V_NEGP
            W()
            i = chain(vector.tensor_scalar(
                out=X.ap()[:, 1:2], in0=negp.ap(), scalar1=rc_ap, scalar2=-1.0,
                op0=Alu.mult, op1=Alu.mult))
            assert i == V_X1
            # epilogue straight from fin PSUM (rd already includes the 0.5)
            vector.wait_ge(tc, T_FIN)
            W()
            i = chain(vector.tensor_scalar(
                out=res.ap()[:, 0:1], in0=fin[:, 0:1], scalar1=nim_ap,
                scalar2=rd_ap, op0=Alu.subtract, op1=Alu.mult))
            assert i == V_RES

        @block.tensor
        def _(tensor):
            tensor.wait_ge(c_sem, 16)
            tensor.transpose(
                out=pvT, in_=pv_ap.to_broadcast([P, P]), identity=ident
            ).then_inc(tc, 1)
            # preload muT's PSUM bank with the (symmetric) additive mask;
            # the mu transpose later accumulates on top of it
            tensor.wait_ge(vc, V_BIGM)
            tensor.matmul(
                out=muT, lhsT=me.ap(), rhs=ident, is_transpose=True,
                start=True, stop=False, skip_group_check=True,
            ).then_inc(tc, 1)
            def mmS(c):
                tensor.wait_ge(sc, S_SQ(c))
                tensor.matmul(
                    out=fmS,
                    lhsT=lh_sb.ap()[:, P * c : P * (c + 1)],
                    rhs=TT.ap()[:, 2 * c + 1 : 2 * c + 2],
                    start=(c == 0),
                    stop=(c == C - 1),
                    skip_group_check=True,
                ).then_inc(tc, 1)

            for c in range(C):
                tensor.wait_ge(tcol[c], 16)
                tensor.matmul(
                    out=fmT,
                    lhsT=lh_sb.ap()[:, P * c : P * (c + 1)],
                    rhs=TT.ap()[:, 2 * c : 2 * c + 1],
                    start=(c == 0),
                    stop=(c == C - 1),
                    skip_group_check=True,
                ).then_inc(tc, 1)
                if c < C - 1:
                    mmS(c)
            # push chain first: mu transpose before the final S2 matmul
            tensor.wait_ge(vc, V_MU)
            tensor.matmul(
                out=muT, lhsT=mu.ap().to_broadcast([P, P]), rhs=ident,
                is_transpose=True, start=False, stop=True,
                skip_group_check=True,
            ).then_inc(tc, 1)
            mmS(C - 1)
            tensor.wait_ge(sc, S_EXP)
            tensor.wait_ge(vc, V_X1)
            tensor.matmul(
                out=fin, lhsT=binds, rhs=X.ap(), start=True, stop=True
            ).then_inc(tc, 1)

        @block.scalar
        def _(scalar):
            # consts load via the ACT HWDGE ring: keeps SP's preamble short so
            # the entry barrier releases earlier (hoisted into the preamble)
            scalar.dma_start(out=c_sb.ap(), in_=consts_d[:]).then_inc(c_sem, 16)
            # first activation triggers the ACT table load; run it at t~0
            scalar.wait_ge(vc, V_WARM)
            scalar.activation(warm.ap(), warm.ap(), Act.Exp)
            for c in range(C):
                scalar.wait_ge(tcol[c], 16)
                scalar.activation(
                    TT.ap()[:, 2 * c + 1 : 2 * c + 2],
                    TT.ap()[:, 2 * c : 2 * c + 1],
                    Act.Square,
                ).then_inc(sc, 1)
            scalar.wait_ge(tc, T_MUT)
            scalar.wait_ge(vc, V_MU)
            scalar.activation(
                d2.ap(), muT, Act.Square, bias=mu.ap(), scale=-1.0
            ).then_inc(sc, 1)
            scalar.wait_ge(sc, S_D2)  # d2 write visible before Exp reads it
            scalar.activation(
                e.ap(), d2.ap(), Act.Exp, scale=-1.0, accum_out=X.ap()[:, 0:1]
            ).then_inc(sc, 1)
            # pull output, then ship the 32B result straight from ACT (HWDGE)
            scalar.wait_ge(tc, T_FIN)
            scalar.activation(
                res.ap()[:, 1:2], fin[:, 1:2], Act.Copy, scale=rn_ap
            ).then_inc(sc, 1)

    ctx.close()
    _hoist_input_dmas(nc)
    nc.compile()
    return nc


def _hoist_input_dmas(nc):
    """Move the gidx DMA (SP) and consts DMA (ACT) from their body blocks
    into the entry block, so both input loads start during engine boot and
    neither trigger delays the entry barrier's critical engine."""
    f = nc.m.functions[0]
    b0 = f.blocks[0]
    for tag, eng in (("_SP_", mybir.EngineType.SP),
                     ("_Activation_", mybir.EngineType.Activation)):
        blk = next(b for b in f.blocks if tag in b.name)
        assert type(blk.instructions[0]).__name__ == "InstDMACopy"
        dma = blk.instructions[0]
        del blk.instructions[0]
        entry = list(b0.instructions)
        pos = min(i for i, inst in enumerate(entry) if inst.engine == eng)
        b0.instructions[pos:pos] = [dma]
    _hoist_first_gathers(nc)


def _hoist_first_gathers(nc):
    """Move the g_sem wait + first two indirect gathers from the gpsimd body
    into the entry block just before the barrier drains, so the Q7 ucode
    IRAM load and the first two descriptor generations overlap the entry
    barrier (everything post-barrier has >1.5us of slack)."""
    f = nc.m.functions[0]
    b0 = f.blocks[0]
    bpool = next(b for b in f.blocks if "_Pool_" in b.name)
    want = ["InstMemset", "InstRegisterMove", "InstEventSemaphore",
            "InstDMACopy", "InstDMACopy"]
    got = [type(inst).__name__ for inst in bpool.instructions[:5]]
    assert got == want, got
    moved = [bpool.instructions[i] for i in range(5)]
    for _ in range(5):
        del bpool.instructions[0]
    pool_drain = next(
        i for i, inst in enumerate(b0.instructions)
        if type(inst).__name__ == "InstDrain"
        and inst.engine == mybir.EngineType.Pool
    )
    assert b0.instructions[pool_drain].sync_info is None
    del b0.instructions[pool_drain]
    entry = list(b0.instructions)
    bar = min(i for i, inst in enumerate(entry)
              if type(inst).__name__ == "InstDrain")
    b0.instructions[bar:bar] = moved


def _prepare_in_maps(tags, joints):
    tags = np.ascontiguousarray(tags, dtype=np.float32).reshape(N, KHW)
    joints = np.asarray(joints)
    idx = joints[..., 0].astype(np.int64)
    visb = joints[..., 1] > 0

    blkid = np.repeat(np.arange(IPC), M)
    bmask = np.zeros((P, P), np.float32)
    bmask[:PPI, :PPI] = (blkid[:, None] == blkid[None, :]).astype(np.float32)
    binds = np.zeros((P, IPC), np.float32)
    for i in range(IPC):
        binds[i * M : (i + 1) * M, i] = 1.0
    ident = np.eye(P, dtype=np.float32)
    iorow = np.tile(np.arange(P, dtype=np.float32), (P, 1))

    in_maps = []
    for cidx in range(NCORES):
        sl = slice(cidx * IPC, (cidx + 1) * IPC)
        vb = visb[sl].reshape(PPI, K)
        cnt = vb.sum(axis=1).astype(np.float32)
        gfull = (
            np.arange(IPC, dtype=np.int64)[:, None, None] * KHW + idx[sl]
        ).reshape(PPI, K)
        pp, kk = np.nonzero(vb)
        vals = gfull[pp, kk]
        S = len(pp)
        assert S <= P * C, f"visible slots {S} > {P * C}"
        s = np.arange(S)
        rows, cols = s % P, s // P
        gidxC = np.full((P, C), 0x7FFF0000, np.int32)  # pads: OOB -> skipped
        gidxC[rows, cols] = vals.astype(np.int32)
        pid = np.full((P, C), 127.0, np.float32)  # pads -> unused person 127
        pid[rows, cols] = pp.astype(np.float32)

        pv = (cnt > 0).astype(np.float32)
        nim = pv.reshape(IPC, M).sum(axis=1)  # [IPC]
        consts = np.zeros((P, CC), np.float32)
        consts[:PPI, RC0] = 1.0 / np.maximum(cnt, 1.0)
        consts[PPI:, RC0] = 1.0
        consts[:, NRC0] = -consts[:, RC0]
        consts[:PPI, PV0] = pv
        consts[:PPI, PVN0] = -BIG * pv
        consts[:IPC, NI0] = nim
        consts[:IPC, RD0] = (0.5 / np.maximum((nim - 1.0) * nim, 1.0)).astype(
            np.float32
        )
        consts[:IPC, RN0] = (1.0 / np.maximum(nim, 1.0)).astype(np.float32)
        consts[:, BM0 : BM0 + P] = bmask
        consts[:, BI0 : BI0 + IPC] = binds
        consts[:, ID0 : ID0 + P] = ident
        consts[:, IO0 : IO0 + P] = iorow
        consts[:, PID0 : PID0 + C] = pid
        in_maps.append(
            {
                "tags": tags[sl].reshape(IPC * KHW, 1),
                "gidx": gidxC,
                "consts": consts,
            }
        )
    return in_maps


_CACHE: dict = {}


def _get_nc():
    if "nc" not in _CACHE:
        _CACHE["nc"] = build_nc()
    return _CACHE["nc"]


def _run(tags, joints, trace=False, **kwargs):
    tags = np.asarray(tags)
    joints = np.asarray(joints)
    nc = _get_nc()
    in_maps = _prepare_in_maps(tags, joints)
    res = run_bass_kernel_spmd(
        nc, in_maps, core_ids=list(range(NCORES)), trace=trace, **kwargs
    )
    outs = np.concatenate(
        [np.asarray(res.results[i]["out"]) for i in range(NCORES)], axis=0
    )  # [N, 2]
    pushes = np.ascontiguousarray(outs[:, 0], dtype=np.float32)
    pulls = np.ascontiguousarray(outs[:, 1], dtype=np.float32)
    return (pushes, pulls), res


def kernel(tags, joints):
    (pushes, pulls), _ = _run(tags, joints, trace=False)
    return pushes, pulls


# revision 4
# speedup vs baseline: 1.0402x; 1.0402x over previous
"""AE (associative embedding) push/pull loss on 8 Trainium2 NeuronCores.

Data-parallel over the batch: core c handles images [4c, 4c+4). Per core the
kernel gathers the visible (person, joint) tag values out of the on-device
4x1114112 tag shard with 9 fixed-shape indirect (SWDGE) DMAs of 128
scattered f32 each (1152 slots; unused slots read tags[0] and are routed to
the unused person row 127, so there are no per-call NEFF variants and no
memsets). Per-person sums of t and t^2 come from 9 accumulating PE matmuls
whose one-hot person-indicator planes are built ON DEVICE by the idle DVE
(tensor_scalar is_equal of a static iota row against a per-slot person-id
column) -- this shrinks the consts DMA from 727KB to ~210KB, which also
removes the SDMA-15 backlog straggler that used to stall the last column.
The push loss builds the block-diagonal 120x120 pairwise exp(-(mu_i-mu_j)^2)
tile via a PE transpose, one ScalarE Square (bias=-mu) and one Exp
(accum_out = row sums), then a final PE matmul against per-image
indicators. The first ScalarE activation (table load) runs at t~0 on a
zeroed scratch, and the 32B result DMA is issued by ScalarE (HWDGE) right
after the last result write instead of hopping back to SP.

Host-side work is index/mask preparation only (no tag data is touched).
Raw bacc (no TileContext): per-engine chain semaphores serialize
same-engine RAW hazards and double as cross-engine handshakes; input DMAs
are hoisted into the preamble region.
"""

from contextlib import ExitStack

import numpy as np

try:
    import concourse  # noqa: F401
except ImportError:
    import sys

    sys.path.insert(0, "/opt/trn_rl_repo")

from concourse import bacc, bass, mybir
from concourse.bass_utils import run_bass_kernel_spmd

N, M, K, KHW = 32, 30, 17, 1114112
NCORES = 8
IPC = N // NCORES
P = 128
PPI = IPC * M
C = 9  # gather columns; 1152 slots >= max visible (~1030, +5.8 sigma)

f32 = mybir.dt.float32
i32 = mybir.dt.int32
Alu = mybir.AluOpType
Act = mybir.ActivationFunctionType

BIG = 1.0e4  # additive mask magnitude: exp(-(BIG+d)^2) underflows to 0

# consts column layout
RC0, NRC0, PV0, PVN0, NI0, RD0, RN0 = 0, 1, 2, 3, 4, 5, 6
BM0 = 7
BI0 = BM0 + P
ID0 = BI0 + IPC
IO0 = ID0 + P
PID0 = IO0 + P
CC = PID0 + C


def build_nc():
    nc = bacc.Bacc(
        "TRN2",
        target_bir_lowering=False,
        debug=False,
        enable_asserts=False,
        num_devices=NCORES,
        detect_race_conditions=False,
    )
    tags_d = nc.declare_dram_parameter("tags", [IPC * KHW, 1], f32, isOutput=False)
    gidx_d = nc.declare_dram_parameter("gidx", [P, C], i32, isOutput=False)
    consts_d = nc.declare_dram_parameter("consts", [P, CC], f32, isOutput=False)
    out_d = nc.declare_dram_parameter("out", [IPC, 2], f32, isOutput=True)

    ctx = ExitStack()
    g_sem = ctx.enter_context(nc.semaphore("g_sem"))
    c_sem = ctx.enter_context(nc.semaphore("c_sem"))
    o_sem = ctx.enter_context(nc.semaphore("o_sem"))
    vc = ctx.enter_context(nc.semaphore("vc_sem"))
    tc = ctx.enter_context(nc.semaphore("tc_sem"))
    sc = ctx.enter_context(nc.semaphore("sc_sem"))
    tcol = [ctx.enter_context(nc.semaphore(f"tcol{c}")) for c in range(C)]

    gidx_sb = ctx.enter_context(nc.sbuf_tensor("gidx_sb", [P, C], i32))
    c_sb = ctx.enter_context(nc.sbuf_tensor("c_sb", [P, CC], f32))
    lh_sb = ctx.enter_context(nc.sbuf_tensor("lh_sb", [P, C * P], f32))
    TT = ctx.enter_context(nc.sbuf_tensor("TT", [P, 2 * C], f32))
    mu = ctx.enter_context(nc.sbuf_tensor("mu", [P, 1], f32))
    t2s = ctx.enter_context(nc.sbuf_tensor("t2s", [P, 1], f32))
    negp = ctx.enter_context(nc.sbuf_tensor("negp", [P, 1], f32))
    X = ctx.enter_context(nc.sbuf_tensor("X", [P, 2], f32))
    d2 = ctx.enter_context(nc.sbuf_tensor("d2", [P, P], f32))
    e = ctx.enter_context(nc.sbuf_tensor("e", [P, P], f32))
    pm = ctx.enter_context(nc.sbuf_tensor("pm", [P, P], f32))
    me = ctx.enter_context(nc.sbuf_tensor("me", [P, P], f32))
    res = ctx.enter_context(nc.sbuf_tensor("res", [IPC, 2], f32))
    warm = ctx.enter_context(nc.sbuf_tensor("warm", [1, 1], f32))
    muT_t = ctx.enter_context(nc.psum_tensor("muT", [P, 512], f32))
    pvT_t = ctx.enter_context(nc.psum_tensor("pvT", [P, 512], f32))
    fmT_t = ctx.enter_context(nc.psum_tensor("fmT", [P, 512], f32))
    fmS_t = ctx.enter_context(nc.psum_tensor("fmS", [P, 512], f32))
    fin_t = ctx.enter_context(nc.psum_tensor("fin", [IPC, 512], f32))

    rc_ap = c_sb.ap()[:, RC0 : RC0 + 1]
    negrc_ap = c_sb.ap()[:, NRC0 : NRC0 + 1]
    pv_ap = c_sb.ap()[:, PV0 : PV0 + 1]
    pvnb_ap = c_sb.ap()[:, PVN0 : PVN0 + 1]
    nim_ap = c_sb.ap()[0:IPC, NI0 : NI0 + 1]
    rd_ap = c_sb.ap()[0:IPC, RD0 : RD0 + 1]
    rn_ap = c_sb.ap()[0:IPC, RN0 : RN0 + 1]
    bmask = c_sb.ap()[:, BM0 : BM0 + P]
    binds = c_sb.ap()[:, BI0 : BI0 + IPC]
    ident = c_sb.ap()[:, ID0 : ID0 + P]
    iorow = c_sb.ap()[:, IO0 : IO0 + P]
    muT = muT_t.ap()[:, :P]
    pvT = pvT_t.ap()[:, :P]
    fmT = fmT_t.ap()[:, :1]
    fmS = fmS_t.ap()[:, :1]
    fin = fin_t.ap()[:, :2]

    V_WARM = 1
    V_LH = lambda b: 2 + b  # b = 0..C-1
    V_PM = 2 + C
    V_BIGM = 3 + C
    V_MU = 4 + C
    V_T2S = 5 + C
    V_NEGP = 6 + C
    V_X1 = 7 + C
    V_RES = 8 + C
    S_SQ = lambda c: 1 + c
    S_D2 = C + 1
    S_EXP = C + 2
    S_RES1 = C + 3

    T_PVT = 1
    T_MMT = lambda c: 3 + 2 * c      # S1 matmul for column c done
    T_MMS = lambda c: 4 + 2 * c if c < C - 1 else 2 * C + 3
    T_MUT = 2 * C + 2                # muT now precedes the last S2 matmul
    T_FIN = 2 * C + 4

    with nc.Block(no_gpsimd_drain=True) as block:

        @block.sync
        def _(sync):
            sync.dma_start(out=gidx_sb.ap(), in_=gidx_d[:]).then_inc(g_sem, 16)
            sync.wait_ge(sc, S_RES1)
            sync.wait_ge(vc, V_RES)
            sync.dma_start(out=out_d[:], in_=res.ap()).then_inc(o_sem, 16)

        @block.gpsimd
        def _(gpsimd):
            gpsimd.memset(TT.ap(), 0.0)
            breg = gpsimd.to_reg(IPC * KHW - 1)
            gpsimd.wait_ge(g_sem, 16)
            for c in range(C):
                kw = (dict(bounds_check=breg, oob_is_err=False)
                      if c == C - 1 else {})
                gpsimd.indirect_dma_start(
                    out=TT.ap()[:, 2 * c : 2 * c + 1],
                    out_offset=None,
                    in_=tags_d[:],
                    in_offset=bass.IndirectOffsetOnAxis(
                        ap=gidx_sb.ap()[:, c : c + 1], axis=0
                    ),
                    **kw,
                ).then_inc(tcol[c], 16)

        @block.vector
        def _(vector):
            vn = {"n": 0}

            def chain(instr):
                instr.then_inc(vc, 1)
                vn["n"] += 1
                return vn["n"]

            def W():
                vector.wait_ge(vc, vn["n"])

            # zero the scalar-engine warm-up scratch at t=0
            i = chain(vector.memset(warm.ap(), 0.0))
            assert i == V_WARM
            # build the one-hot person-indicator planes on device
            vector.wait_ge(c_sem, 16)
            for b in range(C):
                i = chain(vector.tensor_scalar(
                    out=lh_sb.ap()[:, P * b : P * (b + 1)], in0=iorow,
                    scalar1=c_sb.ap()[:, PID0 + b : PID0 + b + 1],
                    scalar2=None, op0=Alu.is_equal))
                assert i == V_LH(b)
            # additive pair mask BIGM = BIG * (1 - pv_p*pv_q*blk)
            vector.wait_ge(tc, T_PVT)
            i = chain(vector.scalar_tensor_tensor(
                out=pm.ap(), in0=pvT, scalar=pvnb_ap, in1=bmask,
                op0=Alu.mult, op1=Alu.mult))
            assert i == V_PM
            W()
            i = chain(vector.tensor_scalar(
                out=me.ap(), in0=pm.ap(), scalar1=BIG, scalar2=None,
                op0=Alu.add))
            assert i == V_BIGM
            # per-person stats straight from PSUM
            vector.wait_ge(tc, T_MMT(C - 1))
            i = chain(vector.tensor_mul(mu.ap(), fmT, rc_ap))
            assert i == V_MU
            vector.wait_ge(tc, T_MMS(C - 1))
            i = chain(vector.tensor_copy(t2s.ap(), fmS))
            assert i == V_T2S
            W()
            i = chain(vector.scalar_tensor_tensor(
                out=negp.ap(), in0=fmT, scalar=mu.ap(), in1=t2s.ap(),
                op0=Alu.mult, op1=Alu.subtract))
            assert i == V_NEGP
            W()
            i = chain(vector.tensor_scalar(
                out=X.ap()[:, 1:2], in0=negp.ap(), scalar1=rc_ap, scalar2=-1.0,
                op0=Alu.mult, op1=Alu.mult))
            assert i == V_X1
            # epilogue straight from fin PSUM (rd already includes the 0.5)
            vector.wait_ge(tc, T_FIN)
            W()
            i = chain(vector.tensor_scalar(
                out=res.ap()[:, 0:1], in0=fin[:, 0:1], scalar1=nim_ap,
                scalar2=rd_ap, op0=Alu.subtract, op1=Alu.mult))
            assert i == V_RES

        @block.tensor
        def _(tensor):
            tensor.wait_ge(c_sem, 16)
            tensor.transpose(
                out=pvT, in_=pv_ap.to_broadcast([P, P]), identity=ident
            ).then_inc(tc, 1)
            # preload muT's PSUM bank with the (symmetric) additive mask;
            # the mu transpose later accumulates on top of it
            tensor.wait_ge(vc, V_BIGM)
            tensor.matmul(
                out=muT, lhsT=me.ap(), rhs=ident, is_transpose=True,
                start=True, stop=False, skip_group_check=True,
            ).then_inc(tc, 1)
            def mmS(c):
                tensor.wait_ge(sc, S_SQ(c))
                tensor.matmul(
                    out=fmS,
                    lhsT=lh_sb.ap()[:, P * c : P * (c + 1)],
                    rhs=TT.ap()[:, 2 * c + 1 : 2 * c + 2],
                    start=(c == 0),
                    stop=(c == C - 1),
                    skip_group_check=True,
                ).then_inc(tc, 1)

            for c in range(C):
                tensor.wait_ge(tcol[c], 16)
                tensor.matmul(
                    out=fmT,
                    lhsT=lh_sb.ap()[:, P * c : P * (c + 1)],
                    rhs=TT.ap()[:, 2 * c : 2 * c + 1],
                    start=(c == 0),
                    stop=(c == C - 1),
                    skip_group_check=True,
                ).then_inc(tc, 1)
                if c < C - 1:
                    mmS(c)
            # push chain first: mu transpose before the final S2 matmul
            tensor.wait_ge(vc, V_MU)
            tensor.matmul(
                out=muT, lhsT=mu.ap().to_broadcast([P, P]), rhs=ident,
                is_transpose=True, start=False, stop=True,
                skip_group_check=True,
            ).then_inc(tc, 1)
            mmS(C - 1)
            tensor.wait_ge(sc, S_EXP)
            tensor.wait_ge(vc, V_X1)
            tensor.matmul(
                out=fin, lhsT=binds, rhs=X.ap(), start=True, stop=True
            ).then_inc(tc, 1)

        @block.scalar
        def _(scalar):
            # consts load via the ACT HWDGE ring: keeps SP's preamble short so
            # the entry barrier releases earlier (hoisted into the preamble)
            scalar.dma_start(out=c_sb.ap(), in_=consts_d[:]).then_inc(c_sem, 16)
            # first activation triggers the ACT table load; run it at t~0
            scalar.wait_ge(vc, V_WARM)
            scalar.activation(warm.ap(), warm.ap(), Act.Exp)
            for c in range(C):
                scalar.wait_ge(tcol[c], 16)
                scalar.activation(
                    TT.ap()[:, 2 * c + 1 : 2 * c + 2],
                    TT.ap()[:, 2 * c : 2 * c + 1],
                    Act.Square,
                ).then_inc(sc, 1)
            scalar.wait_ge(tc, T_MUT)
            scalar.wait_ge(vc, V_MU)
            scalar.activation(
                d2.ap(), muT, Act.Square, bias=mu.ap(), scale=-1.0
            ).then_inc(sc, 1)
            scalar.wait_ge(sc, S_D2)  # d2 write visible before Exp reads it
            scalar.activation(
                e.ap(), d2.ap(), Act.Exp, scale=-1.0, accum_out=X.ap()[:, 0:1]
            ).then_inc(sc, 1)
            # pull output, then ship the 32B result straight from ACT (HWDGE)
            scalar.wait_ge(tc, T_FIN)
            scalar.activation(
                res.ap()[:, 1:2], fin[:, 1:2], Act.Copy, scale=rn_ap
            ).then_inc(sc, 1)

    ctx.close()
    _hoist_input_dmas(nc)
    nc.compile()
    return nc


def _hoist_input_dmas(nc):
    """Move the gidx DMA (SP) and consts DMA (ACT) from their body blocks
    into the entry block, so both input loads start during engine boot and
    neither trigger delays the entry barrier's critical engine."""
    f = nc.m.functions[0]
    b0 = f.blocks[0]
    for tag, eng in (("_SP_", mybir.EngineType.SP),
                     ("_Activation_", mybir.EngineType.Activation)):
        blk = next(b for b in f.blocks if tag in b.name)
        assert type(blk.instructions[0]).__name__ == "InstDMACopy"
        dma = blk.instructions[0]
        del blk.instructions[0]
        entry = list(b0.instructions)
        pos = min(i for i, inst in enumerate(entry) if inst.engine == eng)
        b0.instructions[pos:pos] = [dma]
    _hoist_first_gathers(nc)


def _hoist_first_gathers(nc):
    """Move the g_sem wait + first two indirect gathers from the gpsimd body
    into the entry block just before the barrier drains, so the Q7 ucode
    IRAM load and the first two descriptor generations overlap the entry
    barrier (everything post-barrier has >1.5us of slack)."""
    f = nc.m.functions[0]
    b0 = f.blocks[0]
    bpool = next(b for b in f.blocks if "_Pool_" in b.name)
    want = ["InstMemset", "InstRegisterMove", "InstEventSemaphore",
            "InstDMACopy", "InstDMACopy"]
    got = [type(inst).__name__ for inst in bpool.instructions[:5]]
    assert got == want, got
    moved = [bpool.instructions[i] for i in range(5)]
    for _ in range(5):
        del bpool.instructions[0]
    pool_drain = next(
        i for i, inst in enumerate(b0.instructions)
        if type(inst).__name__ == "InstDrain"
        and inst.engine == mybir.EngineType.Pool
    )
    assert b0.instructions[pool_drain].sync_info is None
    del b0.instructions[pool_drain]
    entry = list(b0.instructions)
    bar = min(i for i, inst in enumerate(entry)
              if type(inst).__name__ == "InstDrain")
    b0.instructions[bar:bar] = moved


def _prepare_in_maps(tags, joints):
    tags = np.ascontiguousarray(tags, dtype=np.float32).reshape(N, KHW)
    joints = np.asarray(joints)
    idx = joints[..., 0].astype(np.int64)
    visb = joints[..., 1] > 0

    blkid = np.repeat(np.arange(IPC), M)
    bmask = np.zeros((P, P), np.float32)
    bmask[:PPI, :PPI] = (blkid[:, None] == blkid[None, :]).astype(np.float32)
    binds = np.zeros((P, IPC), np.float32)
    for i in range(IPC):
        binds[i * M : (i + 1) * M, i] = 1.0
    ident = np.eye(P, dtype=np.float32)
    iorow = np.tile(np.arange(P, dtype=np.float32), (P, 1))

    in_maps = []
    for cidx in range(NCORES):
        sl = slice(cidx * IPC, (cidx + 1) * IPC)
        vb = visb[sl].reshape(PPI, K)
        cnt = vb.sum(axis=1).astype(np.float32)
        gfull = (
            np.arange(IPC, dtype=np.int64)[:, None, None] * KHW + idx[sl]
        ).reshape(PPI, K)
        pp, kk = np.nonzero(vb)
        vals = gfull[pp, kk]
        S = len(pp)
        assert S <= P * C, f"visible slots {S} > {P * C}"
        s = np.arange(S)
        rows, cols = s % P, s // P
        gidxC = np.zeros((P, C), np.int32)  # pads read tags[0] -> person 127
        real = np.zeros((P, C), bool)
        real[rows, cols] = True
        gidxC[rows, cols] = vals.astype(np.int32)
        gidxC[~real[:, C - 1], C - 1] = 0x7FFF0000  # last col pads: skipped
        pid = np.full((P, C), 127.0, np.float32)  # pads -> unused person 127
        pid[rows, cols] = pp.astype(np.float32)

        pv = (cnt > 0).astype(np.float32)
        nim = pv.reshape(IPC, M).sum(axis=1)  # [IPC]
        consts = np.zeros((P, CC), np.float32)
        consts[:PPI, RC0] = 1.0 / np.maximum(cnt, 1.0)
        consts[PPI:, RC0] = 1.0
        consts[:, NRC0] = -consts[:, RC0]
        consts[:PPI, PV0] = pv
        consts[:PPI, PVN0] = -BIG * pv
        consts[:IPC, NI0] = nim
        consts[:IPC, RD0] = (0.5 / np.maximum((nim - 1.0) * nim, 1.0)).astype(
            np.float32
        )
        consts[:IPC, RN0] = (1.0 / np.maximum(nim, 1.0)).astype(np.float32)
        consts[:, BM0 : BM0 + P] = bmask
        consts[:, BI0 : BI0 + IPC] = binds
        consts[:, ID0 : ID0 + P] = ident
        consts[:, IO0 : IO0 + P] = iorow
        consts[:, PID0 : PID0 + C] = pid
        in_maps.append(
            {
                "tags": tags[sl].reshape(IPC * KHW, 1),
                "gidx": gidxC,
                "consts": consts,
            }
        )
    return in_maps


_CACHE: dict = {}


def _get_nc():
    if "nc" not in _CACHE:
        _CACHE["nc"] = build_nc()
    return _CACHE["nc"]


def _run(tags, joints, trace=False, **kwargs):
    tags = np.asarray(tags)
    joints = np.asarray(joints)
    nc = _get_nc()
    in_maps = _prepare_in_maps(tags, joints)
    res = run_bass_kernel_spmd(
        nc, in_maps, core_ids=list(range(NCORES)), trace=trace, **kwargs
    )
    outs = np.concatenate(
        [np.asarray(res.results[i]["out"]) for i in range(NCORES)], axis=0
    )  # [N, 2]
    pushes = np.ascontiguousarray(outs[:, 0], dtype=np.float32)
    pulls = np.ascontiguousarray(outs[:, 1], dtype=np.float32)
    return (pushes, pulls), res


def kernel(tags, joints):
    (pushes, pulls), _ = _run(tags, joints, trace=False)
    return pushes, pulls
